# revision 1
# baseline (speedup 1.0000x reference)
"""Trainium2 Bass kernel for nn_BaselineModel_35175782154746 (dense transformer
block with SiLU attention + relative-position bias).

Sharding: 8 NeuronCores = 4 batches x 2 head-groups (8 heads each).
Each core computes, for its (batch b, head-group g):
    U, Q, K, V projections (columns g*1024:(g+1)*1024 of Wu/Wq/Wk/Wv),
    SiLU attention with rel-pos bias for its 8 heads,
    gated = out * U, partial = gated @ Wf2[g*1024:(g+1)*1024, :].
Host reduces: out[b] = partial[2b] + partial[2b+1] + bf2.

All matmuls run with bf16 operands (fp32 PSUM accumulation) at N=512 moving
dim — the TensorEngine's full-rate path. Layouts keep the contraction dim on
SBUF partitions (inputs pre-transposed on host). The rel-pos bias is added in
PSUM via an identity-matmul of a host-built shifted table (pre-divided by the
attention scale so ACT's native scale finishes scores = silu(scale*(QK+bias));
for the causal variant the mask is folded into that table as -1e5, which silu
maps to an exact 0.0 in fp32). A dense-mask fallback variant handles any
non-causal attn_mask exactly.
"""

import sys
import os

for _p in ("/root/.axon_site/_ro/trn_rl_repo", "/opt/trn_rl_repo"):
    if os.path.isdir(_p) and _p not in sys.path:
        sys.path.append(_p)

import numpy as np

import concourse.bass as bass
import concourse.mybir as mybir
import concourse.tile as tile
from concourse import bacc
from concourse.bass_utils import run_bass_kernel_spmd

B, S, H, NH, MAXLEN = 4, 1024, 2048, 16, 1024
HD = H // NH            # 128
NHL = 8                 # heads per core (local)
HGRP = 2                # head groups
NCORES = 8
KT16 = H // 128         # 16 k-tiles for the H contraction
SCALE = float(HD) ** -0.5

f32 = mybir.dt.float32
f32r = mybir.dt.float32r
bf16 = mybir.dt.bfloat16
SILU = mybir.ActivationFunctionType.Silu
MULT = mybir.AluOpType.mult
ADD = mybir.AluOpType.add

TRACE = False
LAST_EXEC_NS = None
LAST_RES = None
MM_DT = "bf16"          # "bf16" or "f32r" matmul operand dtype
_CACHE = {}


def _build(causal: bool, mm_dt=None):
    mmdt = {"bf16": bf16, "f32r": f32r}[mm_dt or MM_DT]
    nc = bacc.Bacc("TRN2", target_bir_lowering=False, debug=False,
                   num_devices=NCORES)

    def din(name, shape, dt=f32):
        return nc.dram_tensor(name, shape, dt, kind="ExternalInput").ap()

    qT = din("qT", [H, S], mmdt)
    kT = din("kT", [H, S], mmdt)
    vT = din("vT", [H, S], mmdt)
    wq = din("wq", [H, NHL * HD], mmdt)
    wk = din("wk", [H, NHL * HD], mmdt)
    wv = din("wv", [H, NHL * HD], mmdt)
    wu = din("wu", [H, NHL * HD], mmdt)
    wf2 = din("wf2", [NHL * HD, H], mmdt)
    bq = din("bq", [128, NHL])
    bk = din("bk", [128, NHL])
    bu = din("bu", [128, NHL])
    bv = din("bv", [1, NHL * HD], mmdt)
    ones1 = din("ones1", [1, 128], mmdt)
    atab = din("atab", [NHL, 128, 2047], bf16)
    if not causal:
        maskf = din("maskf", [128, NHL, S], bf16)
    out = nc.dram_tensor("out", [S, H], f32, kind="ExternalOutput").ap()

    with tile.TileContext(nc) as tc:
        with (
            tc.tile_pool(name="const", bufs=1) as constp,
            tc.tile_pool(name="gatedp", bufs=1) as gatedp,
        ):
            bq_t = constp.tile([128, NHL], f32, tag="bq")
            bk_t = constp.tile([128, NHL], f32, tag="bk")
            bu_t = constp.tile([128, NHL], f32, tag="bu")
            bv_t = constp.tile([1, NHL * HD], mmdt, tag="bv")
            ones_t = constp.tile([1, 128], mmdt, tag="ones1")

            gatedT = gatedp.tile([128, NHL, S], mmdt, tag="gatedT")
            wf2r = wf2.rearrange("(cb p) n -> p cb n", p=128)

            with tc.tile_pool(name="attres", bufs=1) as attres:
                UT = attres.tile([128, NHL, S], bf16, tag="UT")
                QT = attres.tile([128, NHL, S], mmdt, tag="QT")
                KTt = attres.tile([128, NHL, S], mmdt, tag="KT")
                V = attres.tile([128, NHL, S], mmdt, tag="V")
                at_tiles = [attres.tile([128, 2047], bf16,
                                        tag=f"atab{h}", name=f"atab{h}")
                            for h in range(NHL)]
                if not causal:
                    mask_t = attres.tile([128, NHL, S], bf16, tag="mask")

                with tc.tile_pool(name="inres", bufs=1) as inres:
                    qres = inres.tile([128, KT16, S], mmdt, tag="qres")
                    kres = inres.tile([128, KT16, S], mmdt, tag="kres")
                    # vres shares qres's slot: qres's last read is the Q
                    # phase, V runs last, so the vres load lands during K.
                    vres = inres.tile([128, KT16, S], mmdt, tag="qres",
                                      name="vres")
                    for k in range(KT16):
                        nc.sync.dma_start(qres[:, k, :],
                                          qT[k * 128:(k + 1) * 128, :])
                    nc.sync.dma_start(bu_t[:], bu[:])
                    nc.sync.dma_start(bq_t[:], bq[:])
                    nc.sync.dma_start(bk_t[:], bk[:])
                    nc.sync.dma_start(bv_t[:], bv[:])
                    nc.sync.dma_start(ones_t[:], ones1[:])
                    for k in range(KT16):
                        nc.sync.dma_start(kres[:, k, :],
                                          kT[k * 128:(k + 1) * 128, :])
                    for k in range(KT16):
                        nc.sync.dma_start(vres[:, k, :],
                                          vT[k * 128:(k + 1) * 128, :])
                    for h in range(NHL):
                        nc.sync.dma_start(at_tiles[h][:], atab[h])
                    if not causal:
                        nc.sync.dma_start(mask_t[:], maskf[:])

                    with (
                        tc.tile_pool(name="win", bufs=6 if causal else 4) as winp,
                        tc.tile_pool(name="pps", bufs=1, space="PSUM") as ppsum,
                    ):
                        # ---- projections U, Q, K ([HD, S] transposed) ----
                        for wdram, xres, btile, outtile in (
                            (wu, qres, bu_t, UT),
                            (wq, qres, bq_t, QT),
                            (wk, kres, bk_t, KTt),
                        ):
                            for ih in range(2):
                                ps = [ppsum.tile([128, 512], f32, tag=f"pp{h}",
                                                 name=f"pp{h}")
                                      for h in range(NHL)]
                                for k in range(KT16):
                                    wt = winp.tile([128, NHL * HD], mmdt,
                                                   tag="win")
                                    nc.gpsimd.dma_start(
                                        wt[:], wdram[k * 128:(k + 1) * 128, :])
                                    for h in range(NHL):
                                        nc.tensor.matmul(
                                            ps[h][:],
                                            lhsT=wt[:, h * HD:(h + 1) * HD],
                                            rhs=xres[:, k,
                                                     ih * 512:(ih + 1) * 512],
                                            start=(k == 0),
                                            stop=(k == KT16 - 1))
                                for h in range(NHL):
                                    nc.scalar.activation(
                                        outtile[:, h, ih * 512:(ih + 1) * 512],
                                        ps[h][:], SILU, bias=btile[:, h:h + 1])

                        # ---- projection V (natural layout [S, NHL*HD]) ----
                        for ch in range(2):
                            ps = [ppsum.tile([128, 512], f32, tag=f"pp{sb}",
                                             name=f"ppv{sb}")
                                  for sb in range(8)]
                            for k in range(KT16):
                                wt = winp.tile([128, 512], mmdt, tag="wvin")
                                nc.gpsimd.dma_start(
                                    wt[:], wv[k * 128:(k + 1) * 128,
                                              ch * 512:(ch + 1) * 512])
                                for sb in range(8):
                                    nc.tensor.matmul(
                                        ps[sb][:],
                                        lhsT=vres[:, k, sb * 128:(sb + 1) * 128],
                                        rhs=wt[:],
                                        start=(k == 0), stop=False)
                            for sb in range(8):
                                nc.tensor.matmul(
                                    ps[sb][:],
                                    lhsT=ones_t[:],
                                    rhs=bv_t[:, ch * 512:(ch + 1) * 512],
                                    start=False, stop=True)
                                nc.scalar.activation(
                                    V[:, sb, ch * 512:(ch + 1) * 512],
                                    ps[sb][:], SILU)

                # ---- attention (ih-outer) with f2 sb0-3 interleaved into
                # the ih=1 pass; f2 sb4-7 after ----
                with (
                    tc.tile_pool(name="attnp", bufs=4) as attnp,
                    tc.tile_pool(name="psav", bufs=2, space="PSUM") as psav,
                    tc.tile_pool(name="pssc", bufs=4, space="PSUM") as pssc,
                    tc.tile_pool(name="psf2", bufs=2, space="PSUM") as psf2,
                    tc.tile_pool(name="w2p", bufs=8) as w2p,
                    tc.tile_pool(name="stgp", bufs=3) as stgp,
                ):
                    def emit_attention(h, ih):
                        njb = (4 * ih + 4) if causal else 8
                        at = at_tiles[h]
                        avp = psav.tile([128, 512], f32, tag="av",
                                        name=f"av{h}_{ih}")
                        chunks = [list(range(j, min(j + 2, njb)))
                                  for j in range(0, njb, 2)]
                        att_tiles = {}

                        def emit_scores(ch_):
                            for jb in ch_:
                                scp = pssc.tile([128, 512], f32, tag="sc",
                                                name=f"sc{h}_{ih}_{jb}")
                                nc.tensor.matmul(
                                    scp[:],
                                    lhsT=KTt[:, h, jb * 128:(jb + 1) * 128],
                                    rhs=QT[:, h, ih * 512:(ih + 1) * 512],
                                    start=True, stop=True)
                                att = attnp.tile([128, 512], mmdt, tag="attn",
                                                 name=f"at{h}_{ih}_{jb}")
                                d0 = ih * 512 - jb * 128 + MAXLEN - 1
                                nc.vector.scalar_tensor_tensor(
                                    att[:], scp[:], SCALE, at[:, d0:d0 + 512],
                                    op0=MULT, op1=ADD)
                                nc.scalar.activation(att[:], att[:], SILU)
                                if not causal:
                                    nc.vector.tensor_mul(
                                        att[:], att[:],
                                        mask_t[:, jb, ih * 512:(ih + 1) * 512])
                                att_tiles[jb] = att

                        emit_scores(chunks[0])
                        for ci, ch_ in enumerate(chunks):
                            if ci + 1 < len(chunks):
                                emit_scores(chunks[ci + 1])
                            for jb in ch_:
                                nc.tensor.matmul(
                                    avp[:],
                                    lhsT=V[:, jb, h * HD:(h + 1) * HD],
                                    rhs=att_tiles.pop(jb)[:],
                                    start=(jb == 0), stop=(jb == njb - 1))
                        nc.vector.tensor_mul(
                            gatedT[:, h, ih * 512:(ih + 1) * 512],
                            avp[:],
                            UT[:, h, ih * 512:(ih + 1) * 512])

                    def emit_f2_block(w2t, n, sb):
                        ps = psf2.tile([128, 512], f32, tag="f2",
                                       name=f"f2_{n}_{sb}")
                        for cb in range(NHL):
                            nc.tensor.matmul(
                                ps[:],
                                lhsT=gatedT[:, cb, sb * 128:(sb + 1) * 128],
                                rhs=w2t[:, cb, :],
                                start=(cb == 0), stop=(cb == NHL - 1))
                        st = stgp.tile([128, 512], f32, tag="st",
                                       name=f"st{n}_{sb}")
                        nc.vector.tensor_copy(st[:], ps[:])
                        nc.sync.dma_start(
                            out[sb * 128:(sb + 1) * 128,
                                n * 512:(n + 1) * 512], st[:])

                    for h in range(NHL):
                        emit_attention(h, 0)

                    w2a = []
                    for n in range(4):
                        t = w2p.tile([128, NHL, 512], mmdt, tag="w2",
                                     name=f"w2a{n}")
                        nc.sync.dma_start(t[:],
                                          wf2r[:, :, n * 512:(n + 1) * 512])
                        w2a.append(t)

                    fa = [(n, sb) for n in range(4) for sb in range(4)]
                    w2b = []
                    for i in range(NHL):
                        emit_attention(i, 1)
                        for n, sb in fa[2 * i:2 * (i + 1)]:
                            emit_f2_block(w2a[n], n, sb)
                        if i % 2 == 1:
                            # column i//2 of part A is done - prefetch its
                            # part-B replacement into the freed slot
                            t = w2p.tile([128, NHL, 512], mmdt, tag="w2",
                                         name=f"w2b{i // 2}")
                            nc.gpsimd.dma_start(
                                t[:], wf2r[:, :, (i // 2) * 512:
                                           (i // 2 + 1) * 512])
                            w2b.append(t)

                    for n in range(4):
                        for sb in range(4, 8):
                            emit_f2_block(w2b[n], n, sb)

    nc.compile()
    return nc


def _build_fast(mm_dt=None):
    """Causal-path build: fine-grained causal attention, strip silu,
    V split top/bottom with the bottom interleaved into ih0 attention,
    f2 interleaved into ih1, bf16 partial outputs.

    One PSUM pool with 8 tags (pp0..pp7) is used for the whole kernel so
    bank reuse carries precise per-tag WAR deps instead of pool-close
    barriers: P1 projections use pp0-7, V/f2 accumulators rotate pp0-3,
    attention scores pp4-5, attention AV pp6-7.
    """
    mmdt = {"bf16": bf16, "f32r": f32r}[mm_dt or MM_DT]
    nc = bacc.Bacc("TRN2", target_bir_lowering=False, debug=False,
                   num_devices=NCORES)

    def din(name, shape, dt=f32):
        return nc.dram_tensor(name, shape, dt, kind="ExternalInput").ap()

    qT = din("qT", [H, S], mmdt)
    kT = din("kT", [H, S], mmdt)
    vT = din("vT", [H, S], mmdt)
    wq = din("wq", [H, NHL * HD], mmdt)
    wk = din("wk", [H, NHL * HD], mmdt)
    wv = din("wv", [H, NHL * HD], mmdt)
    wu = din("wu", [H, NHL * HD], mmdt)
    wf2 = din("wf2", [NHL * HD, H], mmdt)
    bq = din("bq", [128, NHL])
    bk = din("bk", [128, NHL])
    bu = din("bu", [128, NHL])
    bvb = din("bvb", [128, NHL * HD], bf16)
    at2d = din("at2", [NHL, 128, 1024], bf16)
    out = nc.dram_tensor("out", [S, H], bf16, kind="ExternalOutput").ap()

    wf2r = wf2.rearrange("(cb p) n -> p cb n", p=128)

    # causal segment tables: (jb, qstart, width, strip_offset)
    def segs_for(ih):
        segs = []
        soff = 0
        q0, q1 = ih * 512, ih * 512 + 512
        for jb in range(8 if ih else 4):
            qs = max(q0, jb * 128)
            w = q1 - qs
            if w <= 0:
                continue
            segs.append((jb, qs, w, soff))
            soff += w
        return segs, soff

    SEGS0, TOT0 = segs_for(0)   # 1280
    SEGS1, TOT1 = segs_for(1)   # 3328

    with tile.TileContext(nc) as tc:
        with (
            tc.tile_pool(name="const", bufs=1) as constp,
            tc.tile_pool(name="attres", bufs=1) as attres,
            tc.tile_pool(name="strips", bufs=2) as stripp,
            tc.tile_pool(name="win2", bufs=14) as winp2,
            tc.tile_pool(name="pps", bufs=1, space="PSUM") as ppsum,
        ):
            bq_t = constp.tile([128, NHL], f32, tag="bq")
            bk_t = constp.tile([128, NHL], f32, tag="bk")
            bu_t = constp.tile([128, NHL], f32, tag="bu")
            bvb_t = constp.tile([128, NHL * HD], bf16, tag="bvb")

            UT = attres.tile([128, NHL, S], bf16, tag="UT")
            QT = attres.tile([128, NHL, S], mmdt, tag="QT")
            KTt = attres.tile([128, NHL, S], mmdt, tag="KT")
            V = attres.tile([128, NHL, S], mmdt, tag="V")
            gatedT = attres.tile([128, NHL, S], mmdt, tag="gatedT")
            at_tiles = [attres.tile([128, 1024], bf16, tag=f"at{h}",
                                    name=f"at{h}")
                        for h in range(NHL)]

            # kv pool: kres slot is reused by vres (WAR dep handled by
            # Tile). K projections run FIRST so kres frees early (~66us)
            # and the vres DMA can land well before the V phase.
            with tc.tile_pool(name="kvp", bufs=1) as kvp:
                kres = kvp.tile([128, KT16, S], mmdt, tag="kres")

                # qres pool is released manually at the end of P3; w2p
                # takes over its SBUF for P4/P5.
                qrp = tc.alloc_tile_pool(name="qrp", bufs=1)
                qres = qrp.tile([128, KT16, S], mmdt, tag="qres")

                with tc.tile_pool(name="win", bufs=9) as winp:
                    wtpre = [winp.tile([128, NHL * HD], mmdt,
                                       tag="win", name=f"wp{i}")
                             for i in range(2)]
                    # critical path first: the first matmul needs only
                    # kres[0, 0:256] and wk[0:128, 0:128]; DMA those tiny
                    # pieces first so it can issue right after the NEFF
                    # preamble (startup DMA bandwidth is heavily
                    # contended across the 8 cores).
                    nc.sync.dma_start(kres[:, 0, 0:256],
                                      kT[0:128, 0:256])
                    nc.gpsimd.dma_start(wtpre[0][:, 0:128],
                                        wk[0:128, 0:128])
                    nc.gpsimd.dma_start(wtpre[0][:, 128:1024],
                                        wk[0:128, 128:1024])
                    nc.sync.dma_start(kres[:, 0, 256:512],
                                      kT[0:128, 256:512])
                    nc.gpsimd.dma_start(wtpre[1][:],
                                        wk[128:256, :])
                    for k in range(1, KT16):
                        nc.sync.dma_start(kres[:, k, 0:512],
                                          kT[k * 128:(k + 1) * 128, 0:512])
                    nc.sync.dma_start(bu_t[:], bu[:])
                    nc.sync.dma_start(bq_t[:], bq[:])
                    nc.sync.dma_start(bk_t[:], bk[:])
                    nc.sync.dma_start(bvb_t[:], bvb[:])
                    for k in range(KT16):
                        nc.sync.dma_start(kres[:, k, 512:1024],
                                          kT[k * 128:(k + 1) * 128,
                                             512:1024])
                    for k in range(KT16):
                        nc.sync.dma_start(qres[:, k, 0:512],
                                          qT[k * 128:(k + 1) * 128, 0:512])
                    for k in range(KT16):
                        nc.sync.dma_start(qres[:, k, 512:1024],
                                          qT[k * 128:(k + 1) * 128,
                                             512:1024])

                    # ---- P1: projections K, U, Q (transposed) ----
                    first = True
                    for wdram, xres, btile, outtile in (
                        (wk, kres, bk_t, KTt),
                        (wu, qres, bu_t, UT),
                        (wq, qres, bq_t, QT),
                    ):
                        for ih in range(2):
                            ps = [ppsum.tile([128, 512], f32,
                                             tag=f"pp{h}", name=f"pp{h}")
                                  for h in range(NHL)]
                            for k in range(KT16):
                                if first and k < 2:
                                    wt = wtpre[k]
                                else:
                                    wt = winp.tile([128, NHL * HD],
                                                   mmdt, tag="win")
                                    nc.gpsimd.dma_start(
                                        wt[:],
                                        wdram[k * 128:(k + 1) * 128, :])
                                for h in range(NHL):
                                    if first and k == 0 and h == 0:
                                        nc.tensor.matmul(
                                            ps[h][:, 0:256],
                                            lhsT=wt[:, 0:HD],
                                            rhs=xres[:, 0, 0:256],
                                            start=True, stop=False)
                                        nc.tensor.matmul(
                                            ps[h][:, 256:512],
                                            lhsT=wt[:, 0:HD],
                                            rhs=xres[:, 0, 256:512],
                                            start=False, stop=False)
                                        continue
                                    nc.tensor.matmul(
                                        ps[h][:],
                                        lhsT=wt[:, h * HD:(h + 1) * HD],
                                        rhs=xres[:, k,
                                                 ih * 512:(ih + 1) * 512],
                                        start=(k == 0),
                                        stop=(k == KT16 - 1))
                            for h in range(NHL):
                                nc.scalar.activation(
                                    outtile[:, h,
                                            ih * 512:(ih + 1) * 512],
                                    ps[h][:], SILU,
                                    bias=btile[:, h:h + 1])
                            first = False

                # vres reuses the kres slot (kres free after block 2).
                vres = kvp.tile([128, KT16, S], mmdt, tag="kres",
                                name="vres")
                for k in range(KT16):
                    nc.sync.dma_start(vres[:, k, :],
                                      vT[k * 128:(k + 1) * 128, :])
                for h in range(NHL):
                    nc.sync.dma_start(at_tiles[h][:], at2d[h])

                # ---- V projection helpers (natural [S, cols]) ----
                def v_pass_tiles(part, ch):
                    # V-top's second channel borrows pp4-7 (idle until
                    # attention starts) so it need not wait for the
                    # first channel's evacuation chain.
                    base = 4 if (part == 0 and ch == 1) else 0
                    return [ppsum.tile([128, 512], f32,
                                       tag=f"pp{base + i}",
                                       name=f"v{part}_{ch}_{i}")
                            for i in range(4)]

                def v_step(part, ps, ch, k):
                    wt = winp2.tile([128, 512], mmdt, tag="wv2")
                    # alternate DMA queues: the sync queue is idle in
                    # this window, doubling the wv feed rate
                    q = nc.sync if k % 2 == 0 else nc.gpsimd
                    q.dma_start(
                        wt[:], wv[k * 128:(k + 1) * 128,
                                  ch * 512:(ch + 1) * 512])
                    for i in range(4):
                        sb = part * 4 + i
                        nc.tensor.matmul(
                            ps[i][:],
                            lhsT=vres[:, k, sb * 128:(sb + 1) * 128],
                            rhs=wt[:],
                            start=(k == 0), stop=(k == KT16 - 1))

                def v_pass_end(part, ps, ch):
                    for i in range(4):
                        sb = part * 4 + i
                        nc.vector.tensor_add(
                            ps[i][:], ps[i][:],
                            bvb_t[:, ch * 512:(ch + 1) * 512])
                        nc.scalar.activation(
                            V[:, sb, ch * 512:(ch + 1) * 512],
                            ps[i][:], SILU)

                # ---- P2: V-top (keys 0..511), dual-queue DMA feed ----
                for ch in range(2):
                    ps = v_pass_tiles(0, ch)
                    for k in range(KT16):
                        v_step(0, ps, ch, k)
                    v_pass_end(0, ps, ch)

                # ---- attention emission helpers ----
                strip_t = {}
                scn = [0]
                avn = [0]

                def emit_scores(h, ih, lo, hi):
                    segs, tot = ((SEGS0, TOT0) if ih == 0
                                 else (SEGS1, TOT1))
                    if lo == 0:
                        strip_t[(h, ih)] = stripp.tile(
                            [128, TOT1], mmdt, tag="strip",
                            name=f"strip{h}_{ih}")
                    strip = strip_t[(h, ih)]
                    for jb, qs, w, soff in segs[lo:hi]:
                        scp = ppsum.tile([128, 512], f32,
                                         tag=f"pp{4 + scn[0] % 2}",
                                         name=f"sc{h}_{ih}_{jb}")
                        scn[0] += 1
                        nc.tensor.matmul(
                            scp[:, 0:w],
                            lhsT=KTt[:, h, jb * 128:(jb + 1) * 128],
                            rhs=QT[:, h, qs:qs + w],
                            start=True, stop=True)
                        d0 = qs - jb * 128
                        nc.vector.scalar_tensor_tensor(
                            strip[:, soff:soff + w], scp[:, 0:w],
                            SCALE, at_tiles[h][:, d0:d0 + w],
                            op0=MULT, op1=ADD)

                def emit_silu(h, ih, a, b):
                    strip = strip_t[(h, ih)]
                    nc.scalar.activation(strip[:, a:b], strip[:, a:b],
                                         SILU)

                def emit_av(h, ih):
                    segs = SEGS0 if ih == 0 else SEGS1
                    strip = strip_t.pop((h, ih))
                    avp = ppsum.tile([128, 512], f32,
                                     tag=f"pp{6 + avn[0] % 2}",
                                     name=f"av{h}_{ih}")
                    avn[0] += 1
                    njb = segs[-1][0]
                    for jb, qs, w, soff in segs:
                        nc.tensor.matmul(
                            avp[:, qs - 512 * ih:qs - 512 * ih + w],
                            lhsT=V[:, jb, h * HD:(h + 1) * HD],
                            rhs=strip[:, soff:soff + w],
                            start=(jb == 0), stop=(jb == njb))
                    nc.vector.tensor_mul(
                        gatedT[:, h, ih * 512:(ih + 1) * 512],
                        avp[:],
                        UT[:, h, ih * 512:(ih + 1) * 512])

                # ---- P3: ih0 attention (lag-1 av) x V-bottom ----
                vsteps = [(ch, k) for ch in range(2)
                          for k in range(KT16)]
                vstate = {"i": 0, "ps": None}

                def vbot_steps(n):
                    for _ in range(n):
                        if vstate["i"] >= len(vsteps):
                            return
                        ch, k = vsteps[vstate["i"]]
                        if k == 0:
                            vstate["ps"] = v_pass_tiles(1, ch)
                        v_step(1, vstate["ps"], ch, k)
                        vstate["i"] += 1
                        if k == KT16 - 1:
                            v_pass_end(1, vstate["ps"], ch)

                for h in range(NHL):
                    emit_scores(h, 0, 0, 2)
                    vbot_steps(1)
                    emit_scores(h, 0, 2, 4)
                    emit_silu(h, 0, 0, 512)
                    emit_silu(h, 0, 512, TOT0)
                    vbot_steps(1)
                    if h > 0:
                        emit_av(h - 1, 0)
                    vbot_steps(2)
                emit_av(NHL - 1, 0)

                # release the qres SBUF; w2p takes it over for P4/P5.
                # The release barrier lands here in each queue stream,
                # when every qres reader has long finished.
                qrp.release()
                w2p = tc.alloc_tile_pool(name="w2p", bufs=4)
                stgp = tc.alloc_tile_pool(name="stgp", bufs=4)

                # w2a loads for f2 (sync queue)
                w2a = []
                for n in range(4):
                    t = w2p.tile([128, NHL, 512], mmdt, tag="w2",
                                 name=f"w2a{n}")
                    nc.sync.dma_start(
                        t[:], wf2r[:, :, n * 512:(n + 1) * 512])
                    w2a.append(t)

                # ---- f2 output block ----
                nf2 = [0]

                def emit_f2_block(w2t, n, sb):
                    ps = ppsum.tile([128, 512], f32,
                                    tag=f"pp{nf2[0] % 4}",
                                    name=f"f2_{n}_{sb}")
                    for cb in range(NHL):
                        nc.tensor.matmul(
                            ps[:],
                            lhsT=gatedT[:, cb,
                                        sb * 128:(sb + 1) * 128],
                            rhs=w2t[:, cb, :],
                            start=(cb == 0), stop=(cb == NHL - 1))
                    st = stgp.tile([128, 512], bf16, tag="st",
                                   name=f"st{n}_{sb}")
                    if nf2[0] % 2 == 1:
                        nc.vector.tensor_copy(st[:], ps[:])
                        oq = nc.sync
                    else:
                        nc.scalar.copy(st[:], ps[:])
                        oq = nc.gpsimd
                    oq.dma_start(
                        out[sb * 128:(sb + 1) * 128,
                            n * 512:(n + 1) * 512], st[:])
                    nf2[0] += 1

                # ---- P4: ih1 attention (lag-1 av) x f2 part A ----
                fa = [(n, sb) for n in range(4) for sb in range(4)]
                w2b = []
                for h in range(NHL):
                    emit_scores(h, 1, 0, 2)
                    emit_f2_block(w2a[fa[2 * h][0]], *fa[2 * h])
                    emit_scores(h, 1, 2, 4)
                    emit_silu(h, 1, 0, 2048)
                    emit_f2_block(w2a[fa[2 * h + 1][0]],
                                  *fa[2 * h + 1])
                    emit_scores(h, 1, 4, 6)
                    if h > 0:
                        emit_av(h - 1, 1)
                    emit_scores(h, 1, 6, 8)
                    emit_silu(h, 1, 2048, TOT1)
                    if h % 2 == 1:
                        t = w2p.tile([128, NHL, 512], mmdt, tag="w2",
                                     name=f"w2b{h // 2}")
                        nc.gpsimd.dma_start(
                            t[:], wf2r[:, :, (h // 2) * 512:
                                       (h // 2 + 1) * 512])
                        w2b.append(t)
                emit_av(NHL - 1, 1)

                # ---- P5: f2 part B ----
                for n in range(4):
                    for sb in range(4, 8):
                        emit_f2_block(w2b[n], n, sb)

                stgp.release()
                w2p.release()

    nc.compile()
    return nc


def _host_shards(query, key, value, attn_mask, Wq, bq, Wk, bk, Wv, bv,
                 Wu, bu, Wf2, rel_table, causal, mm_dt=None):
    """Build the per-core input maps."""
    import ml_dtypes
    npdt = (np.dtype(ml_dtypes.bfloat16) if (mm_dt or MM_DT) == "bf16"
            else np.float32)
    _ONES128 = np.ones((1, 128)).astype(npdt)
    in_maps = []
    # precompute per-head-group weight slices once (shared by 4 cores each)
    gdata = []
    for g in range(HGRP):
        c0, c1 = g * NHL * HD, (g + 1) * NHL * HD
        wq_c = np.ascontiguousarray(Wq[:, c0:c1]).astype(npdt)
        wk_c = np.ascontiguousarray(Wk[:, c0:c1]).astype(npdt)
        wv_c = np.ascontiguousarray(Wv[:, c0:c1]).astype(npdt)
        wu_c = np.ascontiguousarray(Wu[:, c0:c1]).astype(npdt)
        wf2_c = np.ascontiguousarray(Wf2[c0:c1, :]).astype(npdt)
        bq_c = np.ascontiguousarray(bq[c0:c1].reshape(NHL, 128).T)
        bk_c = np.ascontiguousarray(bk[c0:c1].reshape(NHL, 128).T)
        bu_c = np.ascontiguousarray(bu[c0:c1].reshape(NHL, 128).T)
        bv_c = np.ascontiguousarray(bv[c0:c1][None, :]).astype(npdt)
        # atab[h, r, y] = table[y - r, g*NHL + h]; for the causal variant the
        # table is pre-divided by SCALE and masked entries (m < MAXLEN-1,
        # i.e. key index > query index) are -1e5 so silu gives exactly 0.
        y = np.arange(2047)[None, :]
        r = np.arange(128)[:, None]
        idx = y - r                      # [128, 2047]
        valid = (idx >= 0) & (idx <= 2 * MAXLEN - 2)
        idxc = np.clip(idx, 0, 2 * MAXLEN - 2)
        cols = rel_table[:, g * NHL:(g + 1) * NHL]   # [2047, NHL]
        import ml_dtypes as _mld
        if causal:
            cols = np.where(np.arange(2047)[:, None] >= MAXLEN - 1, cols,
                            np.float32(-1e5))
            at = np.where(valid[:, :, None], cols[idxc], np.float32(-1e5))
        else:
            at = cols[idxc] * valid[:, :, None]
        atab_c = np.ascontiguousarray(
            at.transpose(2, 0, 1)).astype(_mld.bfloat16)
        gdata.append((wq_c, wk_c, wv_c, wu_c, wf2_c, bq_c, bk_c, bu_c,
                      bv_c, atab_c))

    for c in range(NCORES):
        b, g = c // HGRP, c % HGRP
        (wq_c, wk_c, wv_c, wu_c, wf2_c, bq_c, bk_c, bu_c, bv_c,
         atab_c) = gdata[g]
        m = {
            "qT": np.ascontiguousarray(query[b].T).astype(npdt),
            "kT": np.ascontiguousarray(key[b].T).astype(npdt),
            "vT": np.ascontiguousarray(value[b].T).astype(npdt),
            "wq": wq_c, "wk": wk_c, "wv": wv_c, "wu": wu_c, "wf2": wf2_c,
            "bq": bq_c, "bk": bk_c, "bu": bu_c, "bv": bv_c, "atab": atab_c,
            "ones1": _ONES128,
        }
        mb = attn_mask[b]
        if not causal:
            import ml_dtypes as _mld
            mf = np.empty((128, NHL, S), _mld.bfloat16)
            for jb in range(8):
                mf[:, jb, :] = mb[:, jb * 128:(jb + 1) * 128].T
            m["maskf"] = mf
        in_maps.append(m)
    return in_maps


def _host_shards_fast(query, key, value, Wq, bq, Wk, bk, Wv, bv,
                      Wu, bu, Wf2, rel_table, mm_dt=None):
    """Per-core input maps for the fast causal build."""
    import ml_dtypes
    npdt = (np.dtype(ml_dtypes.bfloat16) if (mm_dt or MM_DT) == "bf16"
            else np.float32)
    in_maps = []
    gdata = []
    for g in range(HGRP):
        c0, c1 = g * NHL * HD, (g + 1) * NHL * HD
        wq_c = np.ascontiguousarray(Wq[:, c0:c1]).astype(npdt)
        wk_c = np.ascontiguousarray(Wk[:, c0:c1]).astype(npdt)
        wv_c = np.ascontiguousarray(Wv[:, c0:c1]).astype(npdt)
        wu_c = np.ascontiguousarray(Wu[:, c0:c1]).astype(npdt)
        wf2_c = np.ascontiguousarray(Wf2[c0:c1, :]).astype(npdt)
        bq_c = np.ascontiguousarray(bq[c0:c1].reshape(NHL, 128).T)
        bk_c = np.ascontiguousarray(bk[c0:c1].reshape(NHL, 128).T)
        bu_c = np.ascontiguousarray(bu[c0:c1].reshape(NHL, 128).T)
        bvb_c = np.ascontiguousarray(
            np.broadcast_to(bv[c0:c1][None, :], (128, NHL * HD))
        ).astype(ml_dtypes.bfloat16)
        # at2[h, r, y'] = table[y' + MAXLEN-1 - r]; entries with
        # y'+MAXLEN-1-r < MAXLEN-1 (key > query) are -1e5 -> silu = 0.
        y = np.arange(MAXLEN - 1, 2 * MAXLEN - 1)[None, :]   # [1, 1024]
        r = np.arange(128)[:, None]
        idx = y - r                       # [128, 1024], in [896, 2046]
        cols = rel_table[:, g * NHL:(g + 1) * NHL]   # [2047, NHL]
        cols = np.where(np.arange(2 * MAXLEN - 1)[:, None] >= MAXLEN - 1,
                        cols, np.float32(-1e5))
        at2_c = np.ascontiguousarray(
            cols[idx].transpose(2, 0, 1)).astype(ml_dtypes.bfloat16)
        gdata.append((wq_c, wk_c, wv_c, wu_c, wf2_c, bq_c, bk_c, bu_c,
                      bvb_c, at2_c))

    for c in range(NCORES):
        b, g = c // HGRP, c % HGRP
        (wq_c, wk_c, wv_c, wu_c, wf2_c, bq_c, bk_c, bu_c, bvb_c,
         at2_c) = gdata[g]
        in_maps.append({
            "qT": np.ascontiguousarray(query[b].T).astype(npdt),
            "kT": np.ascontiguousarray(key[b].T).astype(npdt),
            "vT": np.ascontiguousarray(value[b].T).astype(npdt),
            "wq": wq_c, "wk": wk_c, "wv": wv_c, "wu": wu_c, "wf2": wf2_c,
            "bq": bq_c, "bk": bk_c, "bu": bu_c, "bvb": bvb_c,
            "at2": at2_c,
        })
    return in_maps


def kernel(query, key, value, attn_mask, Wq, bq, Wk, bk, Wv, bv, Wu, bu,
           Wf2, bf2, rel_table):
    global LAST_EXEC_NS, LAST_RES
    query = np.asarray(query, np.float32)
    key = np.asarray(key, np.float32)
    value = np.asarray(value, np.float32)
    attn_mask = np.asarray(attn_mask, bool)
    Wq, bq = np.asarray(Wq, np.float32), np.asarray(bq, np.float32)
    Wk, bk = np.asarray(Wk, np.float32), np.asarray(bk, np.float32)
    Wv, bv = np.asarray(Wv, np.float32), np.asarray(bv, np.float32)
    Wu, bu = np.asarray(Wu, np.float32), np.asarray(bu, np.float32)
    Wf2, bf2 = np.asarray(Wf2, np.float32), np.asarray(bf2, np.float32)
    rel_table = np.asarray(rel_table, np.float32)

    tril = np.tril(np.ones((S, S), bool))
    causal = all(np.array_equal(attn_mask[b], tril) for b in range(B))

    key_ = (causal, MM_DT)
    if key_ not in _CACHE:
        _CACHE[key_] = _build_fast() if causal else _build(causal)
    nc = _CACHE[key_]

    if causal:
        in_maps = _host_shards_fast(query, key, value, Wq, bq, Wk, bk,
                                    Wv, bv, Wu, bu, Wf2, rel_table)
    else:
        in_maps = _host_shards(query, key, value, attn_mask, Wq, bq, Wk,
                               bk, Wv, bv, Wu, bu, Wf2, rel_table, causal)
    res = run_bass_kernel_spmd(nc, in_maps, list(range(NCORES)), trace=TRACE)
    LAST_RES = res
    if res.exec_time_ns is not None:
        LAST_EXEC_NS = res.exec_time_ns

    outp = np.empty((B, S, H), np.float32)
    for b in range(B):
        outp[b] = (np.asarray(res.results[2 * b]["out"], np.float32)
                   + np.asarray(res.results[2 * b + 1]["out"], np.float32)
                   + bf2[None, :])
    return outp



# revision 12
# speedup vs baseline: 1.1002x; 1.1002x over previous
"""Trainium2 Bass kernel for nn_BaselineModel_35175782154746 (dense transformer
block with SiLU attention + relative-position bias).

Sharding: 8 NeuronCores = 4 batches x 2 head-groups (8 heads each).
Each core computes, for its (batch b, head-group g):
    U, Q, K, V projections (columns g*1024:(g+1)*1024 of Wu/Wq/Wk/Wv),
    SiLU attention with rel-pos bias for its 8 heads,
    gated = out * U, partial = gated @ Wf2[g*1024:(g+1)*1024, :].
Host reduces: out[b] = partial[2b] + partial[2b+1] + bf2.

Matmuls run with bf16 operands (fp32 PSUM accumulation) at N=512 moving
dim, except the Q/K projections which use fp8 e4m3 operands with DoubleRow
perf mode (2 contraction k-tiles per instruction, ~1.5-1.8x bf16
throughput); their quantization noise is attenuated through the
bias-dominated silu attention scores and costs <1e-3 rel-RMS. Layouts keep the contraction dim on
SBUF partitions (inputs pre-transposed on host). The rel-pos bias is added in
PSUM via an identity-matmul of a host-built shifted table (pre-divided by the
attention scale so ACT's native scale finishes scores = silu(scale*(QK+bias));
for the causal variant the mask is folded into that table as -1e5, which silu
maps to an exact 0.0 in fp32). A dense-mask fallback variant handles any
non-causal attn_mask exactly.
"""

import sys
import os

for _p in ("/root/.axon_site/_ro/trn_rl_repo", "/opt/trn_rl_repo"):
    if os.path.isdir(_p) and _p not in sys.path:
        sys.path.append(_p)

import numpy as np

import concourse.bass as bass
import concourse.mybir as mybir
import concourse.tile as tile
from concourse import bacc
from concourse.bass_utils import run_bass_kernel_spmd

B, S, H, NH, MAXLEN = 4, 1024, 2048, 16, 1024
HD = H // NH            # 128
NHL = 8                 # heads per core (local)
HGRP = 2                # head groups
NCORES = 8
KT16 = H // 128         # 16 k-tiles for the H contraction
SCALE = float(HD) ** -0.5

f32 = mybir.dt.float32
f32r = mybir.dt.float32r
bf16 = mybir.dt.bfloat16
f8 = mybir.dt.float8e4
DRMODE = mybir.MatmulPerfMode.DoubleRow
SILU = mybir.ActivationFunctionType.Silu
MULT = mybir.AluOpType.mult
ADD = mybir.AluOpType.add

AW = 4096.0             # fp8 weight pre-scale (W ~ U(-0.022, 0.022))
SC_W = 1.0 / AW

TRACE = False
LAST_EXEC_NS = None
LAST_RES = None
MM_DT = "bf16"          # "bf16" or "f32r" matmul operand dtype
_CACHE = {}


def _build(causal: bool, mm_dt=None):
    mmdt = {"bf16": bf16, "f32r": f32r}[mm_dt or MM_DT]
    nc = bacc.Bacc("TRN2", target_bir_lowering=False, debug=False,
                   num_devices=NCORES)

    def din(name, shape, dt=f32):
        return nc.dram_tensor(name, shape, dt, kind="ExternalInput").ap()

    qT = din("qT", [H, S], mmdt)
    kT = din("kT", [H, S], mmdt)
    vT = din("vT", [H, S], mmdt)
    wq = din("wq", [H, NHL * HD], mmdt)
    wk = din("wk", [H, NHL * HD], mmdt)
    wv = din("wv", [H, NHL * HD], mmdt)
    wu = din("wu", [H, NHL * HD], mmdt)
    wf2 = din("wf2", [NHL * HD, H], mmdt)
    bq = din("bq", [128, NHL])
    bk = din("bk", [128, NHL])
    bu = din("bu", [128, NHL])
    bv = din("bv", [1, NHL * HD], mmdt)
    ones1 = din("ones1", [1, 128], mmdt)
    atab = din("atab", [NHL, 128, 2047], bf16)
    if not causal:
        maskf = din("maskf", [128, NHL, S], bf16)
    out = nc.dram_tensor("out", [S, H], f32, kind="ExternalOutput").ap()

    with tile.TileContext(nc) as tc:
        with (
            tc.tile_pool(name="const", bufs=1) as constp,
            tc.tile_pool(name="gatedp", bufs=1) as gatedp,
        ):
            bq_t = constp.tile([128, NHL], f32, tag="bq")
            bk_t = constp.tile([128, NHL], f32, tag="bk")
            bu_t = constp.tile([128, NHL], f32, tag="bu")
            bv_t = constp.tile([1, NHL * HD], mmdt, tag="bv")
            ones_t = constp.tile([1, 128], mmdt, tag="ones1")

            gatedT = gatedp.tile([128, NHL, S], mmdt, tag="gatedT")
            wf2r = wf2.rearrange("(cb p) n -> p cb n", p=128)

            with tc.tile_pool(name="attres", bufs=1) as attres:
                UT = attres.tile([128, NHL, S], bf16, tag="UT")
                QT = attres.tile([128, NHL, S], mmdt, tag="QT")
                KTt = attres.tile([128, NHL, S], mmdt, tag="KT")
                V = attres.tile([128, NHL, S], mmdt, tag="V")
                at_tiles = [attres.tile([128, 2047], bf16,
                                        tag=f"atab{h}", name=f"atab{h}")
                            for h in range(NHL)]
                if not causal:
                    mask_t = attres.tile([128, NHL, S], bf16, tag="mask")

                with tc.tile_pool(name="inres", bufs=1) as inres:
                    qres = inres.tile([128, KT16, S], mmdt, tag="qres")
                    kres = inres.tile([128, KT16, S], mmdt, tag="kres")
                    # vres shares qres's slot: qres's last read is the Q
                    # phase, V runs last, so the vres load lands during K.
                    vres = inres.tile([128, KT16, S], mmdt, tag="qres",
                                      name="vres")
                    for k in range(KT16):
                        nc.sync.dma_start(qres[:, k, :],
                                          qT[k * 128:(k + 1) * 128, :])
                    nc.sync.dma_start(bu_t[:], bu[:])
                    nc.sync.dma_start(bq_t[:], bq[:])
                    nc.sync.dma_start(bk_t[:], bk[:])
                    nc.sync.dma_start(bv_t[:], bv[:])
                    nc.sync.dma_start(ones_t[:], ones1[:])
                    for k in range(KT16):
                        nc.sync.dma_start(kres[:, k, :],
                                          kT[k * 128:(k + 1) * 128, :])
                    for k in range(KT16):
                        nc.sync.dma_start(vres[:, k, :],
                                          vT[k * 128:(k + 1) * 128, :])
                    for h in range(NHL):
                        nc.sync.dma_start(at_tiles[h][:], atab[h])
                    if not causal:
                        nc.sync.dma_start(mask_t[:], maskf[:])

                    with (
                        tc.tile_pool(name="win", bufs=6 if causal else 4) as winp,
                        tc.tile_pool(name="pps", bufs=1, space="PSUM") as ppsum,
                    ):
                        # ---- projections U, Q, K ([HD, S] transposed) ----
                        for wdram, xres, btile, outtile in (
                            (wu, qres, bu_t, UT),
                            (wq, qres, bq_t, QT),
                            (wk, kres, bk_t, KTt),
                        ):
                            for ih in range(2):
                                ps = [ppsum.tile([128, 512], f32, tag=f"pp{h}",
                                                 name=f"pp{h}")
                                      for h in range(NHL)]
                                for k in range(KT16):
                                    wt = winp.tile([128, NHL * HD], mmdt,
                                                   tag="win")
                                    nc.gpsimd.dma_start(
                                        wt[:], wdram[k * 128:(k + 1) * 128, :])
                                    for h in range(NHL):
                                        nc.tensor.matmul(
                                            ps[h][:],
                                            lhsT=wt[:, h * HD:(h + 1) * HD],
                                            rhs=xres[:, k,
                                                     ih * 512:(ih + 1) * 512],
                                            start=(k == 0),
                                            stop=(k == KT16 - 1))
                                for h in range(NHL):
                                    nc.scalar.activation(
                                        outtile[:, h, ih * 512:(ih + 1) * 512],
                                        ps[h][:], SILU, bias=btile[:, h:h + 1])

                        # ---- projection V (natural layout [S, NHL*HD]) ----
                        for ch in range(2):
                            ps = [ppsum.tile([128, 512], f32, tag=f"pp{sb}",
                                             name=f"ppv{sb}")
                                  for sb in range(8)]
                            for k in range(KT16):
                                wt = winp.tile([128, 512], mmdt, tag="wvin")
                                nc.gpsimd.dma_start(
                                    wt[:], wv[k * 128:(k + 1) * 128,
                                              ch * 512:(ch + 1) * 512])
                                for sb in range(8):
                                    nc.tensor.matmul(
                                        ps[sb][:],
                                        lhsT=vres[:, k, sb * 128:(sb + 1) * 128],
                                        rhs=wt[:],
                                        start=(k == 0), stop=False)
                            for sb in range(8):
                                nc.tensor.matmul(
                                    ps[sb][:],
                                    lhsT=ones_t[:],
                                    rhs=bv_t[:, ch * 512:(ch + 1) * 512],
                                    start=False, stop=True)
                                nc.scalar.activation(
                                    V[:, sb, ch * 512:(ch + 1) * 512],
                                    ps[sb][:], SILU)

                # ---- attention (ih-outer) with f2 sb0-3 interleaved into
                # the ih=1 pass; f2 sb4-7 after ----
                with (
                    tc.tile_pool(name="attnp", bufs=4) as attnp,
                    tc.tile_pool(name="psav", bufs=2, space="PSUM") as psav,
                    tc.tile_pool(name="pssc", bufs=4, space="PSUM") as pssc,
                    tc.tile_pool(name="psf2", bufs=2, space="PSUM") as psf2,
                    tc.tile_pool(name="w2p", bufs=8) as w2p,
                    tc.tile_pool(name="stgp", bufs=3) as stgp,
                ):
                    def emit_attention(h, ih):
                        njb = (4 * ih + 4) if causal else 8
                        at = at_tiles[h]
                        avp = psav.tile([128, 512], f32, tag="av",
                                        name=f"av{h}_{ih}")
                        chunks = [list(range(j, min(j + 2, njb)))
                                  for j in range(0, njb, 2)]
                        att_tiles = {}

                        def emit_scores(ch_):
                            for jb in ch_:
                                scp = pssc.tile([128, 512], f32, tag="sc",
                                                name=f"sc{h}_{ih}_{jb}")
                                nc.tensor.matmul(
                                    scp[:],
                                    lhsT=KTt[:, h, jb * 128:(jb + 1) * 128],
                                    rhs=QT[:, h, ih * 512:(ih + 1) * 512],
                                    start=True, stop=True)
                                att = attnp.tile([128, 512], mmdt, tag="attn",
                                                 name=f"at{h}_{ih}_{jb}")
                                d0 = ih * 512 - jb * 128 + MAXLEN - 1
                                nc.vector.scalar_tensor_tensor(
                                    att[:], scp[:], SCALE, at[:, d0:d0 + 512],
                                    op0=MULT, op1=ADD)
                                nc.scalar.activation(att[:], att[:], SILU)
                                if not causal:
                                    nc.vector.tensor_mul(
                                        att[:], att[:],
                                        mask_t[:, jb, ih * 512:(ih + 1) * 512])
                                att_tiles[jb] = att

                        emit_scores(chunks[0])
                        for ci, ch_ in enumerate(chunks):
                            if ci + 1 < len(chunks):
                                emit_scores(chunks[ci + 1])
                            for jb in ch_:
                                nc.tensor.matmul(
                                    avp[:],
                                    lhsT=V[:, jb, h * HD:(h + 1) * HD],
                                    rhs=att_tiles.pop(jb)[:],
                                    start=(jb == 0), stop=(jb == njb - 1))
                        nc.vector.tensor_mul(
                            gatedT[:, h, ih * 512:(ih + 1) * 512],
                            avp[:],
                            UT[:, h, ih * 512:(ih + 1) * 512])

                    def emit_f2_block(w2t, n, sb):
                        ps = psf2.tile([128, 512], f32, tag="f2",
                                       name=f"f2_{n}_{sb}")
                        for cb in range(NHL):
                            nc.tensor.matmul(
                                ps[:],
                                lhsT=gatedT[:, cb, sb * 128:(sb + 1) * 128],
                                rhs=w2t[:, cb, :],
                                start=(cb == 0), stop=(cb == NHL - 1))
                        st = stgp.tile([128, 512], f32, tag="st",
                                       name=f"st{n}_{sb}")
                        nc.vector.tensor_copy(st[:], ps[:])
                        nc.sync.dma_start(
                            out[sb * 128:(sb + 1) * 128,
                                n * 512:(n + 1) * 512], st[:])

                    for h in range(NHL):
                        emit_attention(h, 0)

                    w2a = []
                    for n in range(4):
                        t = w2p.tile([128, NHL, 512], mmdt, tag="w2",
                                     name=f"w2a{n}")
                        nc.sync.dma_start(t[:],
                                          wf2r[:, :, n * 512:(n + 1) * 512])
                        w2a.append(t)

                    fa = [(n, sb) for n in range(4) for sb in range(4)]
                    w2b = []
                    for i in range(NHL):
                        emit_attention(i, 1)
                        for n, sb in fa[2 * i:2 * (i + 1)]:
                            emit_f2_block(w2a[n], n, sb)
                        if i % 2 == 1:
                            # column i//2 of part A is done - prefetch its
                            # part-B replacement into the freed slot
                            t = w2p.tile([128, NHL, 512], mmdt, tag="w2",
                                         name=f"w2b{i // 2}")
                            nc.gpsimd.dma_start(
                                t[:], wf2r[:, :, (i // 2) * 512:
                                           (i // 2 + 1) * 512])
                            w2b.append(t)

                    for n in range(4):
                        for sb in range(4, 8):
                            emit_f2_block(w2b[n], n, sb)

    nc.compile()
    return nc


def _build_fast(mm_dt=None):
    """Causal-path build: fine-grained causal attention, strip silu,
    V split top/bottom with the bottom interleaved into ih0 attention,
    f2 interleaved into ih1, bf16 partial outputs.

    Q and K projections run in fp8 e4m3 with DoubleRow perf mode (K-pairs
    of 128 contracted per matmul): inputs qT8/kT8 are e4m3 casts, weights
    wq8/wk8 are e4m3 of AW*W with the 1/AW folded into the SILU
    activation's scale. Numerically this adds <1e-3 rel-RMS (the Q/K
    quantization noise is attenuated through the bias-dominated silu
    scores), while U/V/f2 stay bf16 (their quantization hits the output
    multiplicatively and would blow the error budget).

    One PSUM pool with 8 tags (pp0..pp7) is used for the whole kernel so
    bank reuse carries precise per-tag WAR deps instead of pool-close
    barriers: P1 projections use pp0-7, V/f2 accumulators rotate pp0-3,
    attention scores pp4-5, attention AV pp6-7.
    """
    mmdt = {"bf16": bf16, "f32r": f32r}[mm_dt or MM_DT]
    nc = bacc.Bacc("TRN2", target_bir_lowering=False, debug=False,
                   num_devices=NCORES)

    def din(name, shape, dt=f32):
        return nc.dram_tensor(name, shape, dt, kind="ExternalInput").ap()

    qT = din("qT", [H, S], mmdt)
    qT8 = din("qT8", [H, S], f8)
    kT8 = din("kT8", [H, S], f8)
    vT = din("vT", [H, S], mmdt)
    wq8 = din("wq8", [H, NHL * HD], f8)
    wk8 = din("wk8", [H, NHL * HD], f8)
    wv = din("wv", [H, NHL * HD], mmdt)
    wu = din("wu", [H, NHL * HD], mmdt)
    wf2 = din("wf2", [NHL * HD, H], mmdt)
    bq = din("bq", [128, NHL])
    bk = din("bk", [128, NHL])
    bu = din("bu", [128, NHL])
    bvb = din("bvb", [128, NHL * HD], bf16)
    at2d = din("at2", [NHL, 128, 1024], bf16)
    out = nc.dram_tensor("out", [S, H], bf16, kind="ExternalOutput").ap()

    wf2r = wf2.rearrange("(cb p) n -> p cb n", p=128)
    qT8r = qT8.rearrange("(kp two p) s -> kp p two s", two=2, p=128)
    kT8r = kT8.rearrange("(kp two p) s -> kp p two s", two=2, p=128)
    wq8r = wq8.rearrange("(kp two p) n -> kp p two n", two=2, p=128)
    wk8r = wk8.rearrange("(kp two p) n -> kp p two n", two=2, p=128)

    # causal segment tables: (jb, qstart, width, strip_offset)
    def segs_for(ih):
        segs = []
        soff = 0
        q0, q1 = ih * 512, ih * 512 + 512
        for jb in range(8 if ih else 4):
            qs = max(q0, jb * 128)
            w = q1 - qs
            if w <= 0:
                continue
            segs.append((jb, qs, w, soff))
            soff += w
        return segs, soff

    SEGS0, TOT0 = segs_for(0)   # 1280
    SEGS1, TOT1 = segs_for(1)   # 3328

    with tile.TileContext(nc) as tc:
        with (
            tc.tile_pool(name="const", bufs=1) as constp,
            tc.tile_pool(name="attres", bufs=1) as attres,
            tc.tile_pool(name="strips", bufs=2) as stripp,
            tc.tile_pool(name="win2", bufs=14) as winp2,
            tc.tile_pool(name="wpair", bufs=6) as wpairp,
            tc.tile_pool(name="pps", bufs=1, space="PSUM") as ppsum,
        ):
            bq_t = constp.tile([128, NHL], f32, tag="bq")
            bk_t = constp.tile([128, NHL], f32, tag="bk")
            bu_t = constp.tile([128, NHL], f32, tag="bu")
            bvb_t = constp.tile([128, NHL * HD], bf16, tag="bvb")

            UT = attres.tile([128, NHL, S], bf16, tag="UT")
            QT = attres.tile([128, NHL, S], mmdt, tag="QT")
            KTt = attres.tile([128, NHL, S], mmdt, tag="KT")
            V = attres.tile([128, NHL, S], mmdt, tag="V")
            at_tiles = [attres.tile([128, 1024], bf16, tag=f"at{h}",
                                    name=f"at{h}")
                        for h in range(NHL)]

            # kv pool: the fp8 kres8 slot (16KB/part) is reused by gatedT
            # (bf16, same 16KB) once the K projection has consumed it.
            with tc.tile_pool(name="kvp", bufs=1) as kvp:
                kres8 = kvp.tile([128, KT16, S], f8, tag="kres")

                # qres/qres8 pools are released manually at the end of
                # P3; w2p/stgp take over their SBUF for P4/P5. vres
                # reuses qres's slot (WAR dep handled by Tile).
                qrp = tc.alloc_tile_pool(name="qrp", bufs=1)
                qres = qrp.tile([128, KT16, S], mmdt, tag="qres")
                q8p = tc.alloc_tile_pool(name="q8p", bufs=1)
                qres8 = q8p.tile([128, KT16, S], f8, tag="q8")

                with tc.tile_pool(name="win", bufs=6) as winp:
                    # critical path first: the first DR matmul needs only
                    # kres8[0:2 pairs, cols 0:512] and the first wk8 pair
                    # tile; DMA those first so the PE can start right
                    # after the NEFF preamble.
                    wt0 = wpairp.tile([128, 2, 512], f8, tag="wpair",
                                      name="wt0")
                    nc.sync.dma_start(kres8[:, 0:2, 0:512],
                                      kT8r[0][:, :, 0:512])
                    nc.gpsimd.dma_start(wt0[:, :, 0:128],
                                        wk8r[0][:, :, 0:128])
                    nc.gpsimd.dma_start(wt0[:, :, 128:512],
                                        wk8r[0][:, :, 128:512])
                    nc.sync.dma_start(kres8[:, 0:2, 512:1024],
                                      kT8r[0][:, :, 512:1024])
                    for kp in range(1, 8):
                        nc.sync.dma_start(kres8[:, 2 * kp:2 * kp + 2, :],
                                          kT8r[kp])
                    nc.sync.dma_start(bu_t[:], bu[:])
                    nc.sync.dma_start(bq_t[:], bq[:])
                    nc.sync.dma_start(bk_t[:], bk[:])
                    nc.sync.dma_start(bvb_t[:], bvb[:])
                    for k in range(KT16):
                        nc.sync.dma_start(qres[:, k, 0:512],
                                          qT[k * 128:(k + 1) * 128, 0:512])
                    for k in range(KT16):
                        nc.sync.dma_start(qres[:, k, 512:1024],
                                          qT[k * 128:(k + 1) * 128,
                                             512:1024])
                    for kp in range(8):
                        nc.sync.dma_start(qres8[:, 2 * kp:2 * kp + 2, :],
                                          qT8r[kp])

                    # ---- P1 fp8 projection: 4 heads x 2 ih windows per
                    # half so each DoubleRow weight tile feeds 2 matmuls
                    # (amortizes the non-FWL 256-col LDWEIGHTS).
                    def emit_proj_fp8(w8r, x8, btile, outtile,
                                      first=False):
                        for half in range(2):
                            ps = [ppsum.tile([128, 512], f32,
                                             tag=f"pp{i}", name=f"pp{i}")
                                  for i in range(8)]
                            for kp in range(8):
                                if first and half == 0 and kp == 0:
                                    wt = wt0
                                else:
                                    wt = wpairp.tile([128, 2, 512], f8,
                                                     tag="wpair")
                                    nc.gpsimd.dma_start(
                                        wt[:],
                                        w8r[kp][:, :, half * 512:
                                                (half + 1) * 512])
                                for hh in range(4):
                                    lhsT = wt[:, :, hh * 128:
                                              (hh + 1) * 128]
                                    for ih in range(2):
                                        nc.tensor.matmul(
                                            ps[hh * 2 + ih][:],
                                            lhsT=lhsT,
                                            rhs=x8[:, 2 * kp:2 * kp + 2,
                                                   ih * 512:
                                                   (ih + 1) * 512],
                                            start=(kp == 0),
                                            stop=(kp == 7),
                                            perf_mode=DRMODE)
                            for hh in range(4):
                                h = half * 4 + hh
                                for ih in range(2):
                                    nc.scalar.activation(
                                        outtile[:, h,
                                                ih * 512:(ih + 1) * 512],
                                        ps[hh * 2 + ih][:], SILU,
                                        bias=btile[:, h:h + 1],
                                        scale=SC_W)

                    # ---- P1: K (fp8), U (bf16), Q (fp8) ----
                    emit_proj_fp8(wk8r, kres8, bk_t, KTt, first=True)

                    for ih in range(2):
                        ps = [ppsum.tile([128, 512], f32,
                                         tag=f"pp{h}", name=f"pp{h}")
                              for h in range(NHL)]
                        for k in range(KT16):
                            wt = winp.tile([128, NHL * HD],
                                           mmdt, tag="win")
                            nc.gpsimd.dma_start(
                                wt[:],
                                wu[k * 128:(k + 1) * 128, :])
                            for h in range(NHL):
                                nc.tensor.matmul(
                                    ps[h][:],
                                    lhsT=wt[:, h * HD:(h + 1) * HD],
                                    rhs=qres[:, k,
                                             ih * 512:(ih + 1) * 512],
                                    start=(k == 0),
                                    stop=(k == KT16 - 1))
                        for h in range(NHL):
                            nc.scalar.activation(
                                UT[:, h, ih * 512:(ih + 1) * 512],
                                ps[h][:], SILU,
                                bias=bu_t[:, h:h + 1])

                    emit_proj_fp8(wq8r, qres8, bq_t, QT)

                # vres reuses the qres slot (qres free after U-proj).
                vres = qrp.tile([128, KT16, S], mmdt, tag="qres",
                                name="vres")
                for k in range(KT16):
                    nc.sync.dma_start(vres[:, k, :],
                                      vT[k * 128:(k + 1) * 128, :])
                for h in range(NHL):
                    nc.sync.dma_start(at_tiles[h][:], at2d[h])
                # gatedT takes over kres8's slot (free after K-proj).
                gatedT = kvp.tile([128, NHL, S], mmdt, tag="kres",
                                  name="gatedT")

                # ---- V projection helpers (natural [S, cols]) ----
                def v_pass_tiles(part, ch):
                    # V-top's second channel borrows pp4-7 (idle until
                    # attention starts) so it need not wait for the
                    # first channel's evacuation chain.
                    base = 4 if (part == 0 and ch == 1) else 0
                    return [ppsum.tile([128, 512], f32,
                                       tag=f"pp{base + i}",
                                       name=f"v{part}_{ch}_{i}")
                            for i in range(4)]

                def v_step(part, ps, ch, k):
                    wt = winp2.tile([128, 512], mmdt, tag="wv2")
                    # alternate DMA queues to double the wv feed rate.
                    # In P2 the sync queue is draining vres+atab (vres
                    # waits for the U projection to free qres's slot),
                    # so V-top uses the mostly-idle scalar queue instead.
                    if part == 0:
                        q = nc.scalar if k % 2 == 0 else nc.gpsimd
                    else:
                        q = nc.sync if k % 2 == 0 else nc.gpsimd
                    q.dma_start(
                        wt[:], wv[k * 128:(k + 1) * 128,
                                  ch * 512:(ch + 1) * 512])
                    for i in range(4):
                        sb = part * 4 + i
                        nc.tensor.matmul(
                            ps[i][:],
                            lhsT=vres[:, k, sb * 128:(sb + 1) * 128],
                            rhs=wt[:],
                            start=(k == 0), stop=(k == KT16 - 1))

                def v_pass_end(part, ps, ch):
                    for i in range(4):
                        sb = part * 4 + i
                        nc.vector.tensor_add(
                            ps[i][:], ps[i][:],
                            bvb_t[:, ch * 512:(ch + 1) * 512])
                        nc.scalar.activation(
                            V[:, sb, ch * 512:(ch + 1) * 512],
                            ps[i][:], SILU)

                # ---- P2: V-top (keys 0..511), dual-queue DMA feed ----
                for ch in range(2):
                    ps = v_pass_tiles(0, ch)
                    for k in range(KT16):
                        v_step(0, ps, ch, k)
                    v_pass_end(0, ps, ch)

                # ---- attention emission helpers ----
                strip_t = {}
                scn = [0]
                avn = [0]

                def emit_scores(h, ih, lo, hi):
                    segs, tot = ((SEGS0, TOT0) if ih == 0
                                 else (SEGS1, TOT1))
                    if lo == 0:
                        strip_t[(h, ih)] = stripp.tile(
                            [128, TOT1], mmdt, tag="strip",
                            name=f"strip{h}_{ih}")
                    strip = strip_t[(h, ih)]
                    for jb, qs, w, soff in segs[lo:hi]:
                        scp = ppsum.tile([128, 512], f32,
                                         tag=f"pp{4 + scn[0] % 2}",
                                         name=f"sc{h}_{ih}_{jb}")
                        scn[0] += 1
                        nc.tensor.matmul(
                            scp[:, 0:w],
                            lhsT=KTt[:, h, jb * 128:(jb + 1) * 128],
                            rhs=QT[:, h, qs:qs + w],
                            start=True, stop=True)
                        d0 = qs - jb * 128
                        nc.vector.scalar_tensor_tensor(
                            strip[:, soff:soff + w], scp[:, 0:w],
                            SCALE, at_tiles[h][:, d0:d0 + w],
                            op0=MULT, op1=ADD)

                def emit_silu(h, ih, a, b):
                    strip = strip_t[(h, ih)]
                    nc.scalar.activation(strip[:, a:b], strip[:, a:b],
                                         SILU)

                def emit_av(h, ih):
                    segs = SEGS0 if ih == 0 else SEGS1
                    strip = strip_t.pop((h, ih))
                    avp = ppsum.tile([128, 512], f32,
                                     tag=f"pp{6 + avn[0] % 2}",
                                     name=f"av{h}_{ih}")
                    avn[0] += 1
                    njb = segs[-1][0]
                    for jb, qs, w, soff in segs:
                        nc.tensor.matmul(
                            avp[:, qs - 512 * ih:qs - 512 * ih + w],
                            lhsT=V[:, jb, h * HD:(h + 1) * HD],
                            rhs=strip[:, soff:soff + w],
                            start=(jb == 0), stop=(jb == njb))
                    nc.vector.tensor_mul(
                        gatedT[:, h, ih * 512:(ih + 1) * 512],
                        avp[:],
                        UT[:, h, ih * 512:(ih + 1) * 512])

                # ---- P3: ih0 attention (lag-1 av) x V-bottom ----
                vsteps = [(ch, k) for ch in range(2)
                          for k in range(KT16)]
                vstate = {"i": 0, "ps": None}

                def vbot_steps(n):
                    for _ in range(n):
                        if vstate["i"] >= len(vsteps):
                            return
                        ch, k = vsteps[vstate["i"]]
                        if k == 0:
                            vstate["ps"] = v_pass_tiles(1, ch)
                        v_step(1, vstate["ps"], ch, k)
                        vstate["i"] += 1
                        if k == KT16 - 1:
                            v_pass_end(1, vstate["ps"], ch)

                # front-load V-bottom so vres's last read (and the pool
                # release barrier gating the w2a loads) lands ~2
                # h-iterations before P3 ends.
                VBUD = [6, 6, 6, 6, 4, 4, 0, 0]
                for h in range(NHL):
                    emit_scores(h, 0, 0, 2)
                    vbot_steps(min(2, VBUD[h]))
                    emit_scores(h, 0, 2, 4)
                    emit_silu(h, 0, 0, 512)
                    emit_silu(h, 0, 512, TOT0)
                    vbot_steps(min(2, max(0, VBUD[h] - 2)))
                    if h > 0:
                        emit_av(h - 1, 0)
                    vbot_steps(max(0, VBUD[h] - 4))
                emit_av(NHL - 1, 0)

                # release the qres(->vres) and qres8 SBUF; w2p/stgp take
                # it over for P4/P5. The release barrier lands here in
                # each queue stream, when every reader has finished.
                q8p.release()
                qrp.release()
                w2p = tc.alloc_tile_pool(name="w2p", bufs=4)
                stgp = tc.alloc_tile_pool(name="stgp", bufs=4)

                # w2a loads for f2 (sync queue)
                w2a = []
                for n in range(4):
                    t = w2p.tile([128, NHL, 512], mmdt, tag="w2",
                                 name=f"w2a{n}")
                    nc.sync.dma_start(
                        t[:], wf2r[:, :, n * 512:(n + 1) * 512])
                    w2a.append(t)

                # ---- f2 output block ----
                nf2 = [0]

                def emit_f2_block(w2t, n, sb):
                    ps = ppsum.tile([128, 512], f32,
                                    tag=f"pp{nf2[0] % 4}",
                                    name=f"f2_{n}_{sb}")
                    for cb in range(NHL):
                        nc.tensor.matmul(
                            ps[:],
                            lhsT=gatedT[:, cb,
                                        sb * 128:(sb + 1) * 128],
                            rhs=w2t[:, cb, :],
                            start=(cb == 0), stop=(cb == NHL - 1))
                    st = stgp.tile([128, 512], bf16, tag="st",
                                   name=f"st{n}_{sb}")
                    if nf2[0] % 2 == 1:
                        nc.vector.tensor_copy(st[:], ps[:])
                        oq = nc.sync
                    else:
                        nc.scalar.copy(st[:], ps[:])
                        oq = nc.gpsimd
                    oq.dma_start(
                        out[sb * 128:(sb + 1) * 128,
                            n * 512:(n + 1) * 512], st[:])
                    nf2[0] += 1

                # ---- P4: ih1 attention (lag-1 av) x f2 part A ----
                # h=0 runs no f2 so the post-release w2a[0] DMA has a
                # full h-iteration to land; later h catch up.
                fa = [(n, sb) for n in range(4) for sb in range(4)]
                F2SCHED = [0, 3, 3, 2, 2, 2, 2, 2]
                fidx = [0]

                def emit_f2a(nblk):
                    for _ in range(nblk):
                        n, sb = fa[fidx[0]]
                        emit_f2_block(w2a[n], n, sb)
                        fidx[0] += 1

                w2b = []
                for h in range(NHL):
                    c = F2SCHED[h]
                    emit_scores(h, 1, 0, 2)
                    emit_f2a(c // 2)
                    emit_scores(h, 1, 2, 4)
                    emit_silu(h, 1, 0, 2048)
                    emit_f2a(c - c // 2)
                    emit_scores(h, 1, 4, 6)
                    if h > 0:
                        emit_av(h - 1, 1)
                    emit_scores(h, 1, 6, 8)
                    emit_silu(h, 1, 2048, TOT1)
                    if h % 2 == 1:
                        t = w2p.tile([128, NHL, 512], mmdt, tag="w2",
                                     name=f"w2b{h // 2}")
                        nc.gpsimd.dma_start(
                            t[:], wf2r[:, :, (h // 2) * 512:
                                       (h // 2 + 1) * 512])
                        w2b.append(t)
                emit_av(NHL - 1, 1)

                # ---- P5: f2 part B ----
                for n in range(4):
                    for sb in range(4, 8):
                        emit_f2_block(w2b[n], n, sb)

                stgp.release()
                w2p.release()

    nc.compile()
    return nc


def _host_shards(query, key, value, attn_mask, Wq, bq, Wk, bk, Wv, bv,
                 Wu, bu, Wf2, rel_table, causal, mm_dt=None):
    """Build the per-core input maps."""
    import ml_dtypes
    npdt = (np.dtype(ml_dtypes.bfloat16) if (mm_dt or MM_DT) == "bf16"
            else np.float32)
    _ONES128 = np.ones((1, 128)).astype(npdt)
    in_maps = []
    # precompute per-head-group weight slices once (shared by 4 cores each)
    gdata = []
    for g in range(HGRP):
        c0, c1 = g * NHL * HD, (g + 1) * NHL * HD
        wq_c = np.ascontiguousarray(Wq[:, c0:c1]).astype(npdt)
        wk_c = np.ascontiguousarray(Wk[:, c0:c1]).astype(npdt)
        wv_c = np.ascontiguousarray(Wv[:, c0:c1]).astype(npdt)
        wu_c = np.ascontiguousarray(Wu[:, c0:c1]).astype(npdt)
        wf2_c = np.ascontiguousarray(Wf2[c0:c1, :]).astype(npdt)
        bq_c = np.ascontiguousarray(bq[c0:c1].reshape(NHL, 128).T)
        bk_c = np.ascontiguousarray(bk[c0:c1].reshape(NHL, 128).T)
        bu_c = np.ascontiguousarray(bu[c0:c1].reshape(NHL, 128).T)
        bv_c = np.ascontiguousarray(bv[c0:c1][None, :]).astype(npdt)
        # atab[h, r, y] = table[y - r, g*NHL + h]; for the causal variant the
        # table is pre-divided by SCALE and masked entries (m < MAXLEN-1,
        # i.e. key index > query index) are -1e5 so silu gives exactly 0.
        y = np.arange(2047)[None, :]
        r = np.arange(128)[:, None]
        idx = y - r                      # [128, 2047]
        valid = (idx >= 0) & (idx <= 2 * MAXLEN - 2)
        idxc = np.clip(idx, 0, 2 * MAXLEN - 2)
        cols = rel_table[:, g * NHL:(g + 1) * NHL]   # [2047, NHL]
        import ml_dtypes as _mld
        if causal:
            cols = np.where(np.arange(2047)[:, None] >= MAXLEN - 1, cols,
                            np.float32(-1e5))
            at = np.where(valid[:, :, None], cols[idxc], np.float32(-1e5))
        else:
            at = cols[idxc] * valid[:, :, None]
        atab_c = np.ascontiguousarray(
            at.transpose(2, 0, 1)).astype(_mld.bfloat16)
        gdata.append((wq_c, wk_c, wv_c, wu_c, wf2_c, bq_c, bk_c, bu_c,
                      bv_c, atab_c))

    for c in range(NCORES):
        b, g = c // HGRP, c % HGRP
        (wq_c, wk_c, wv_c, wu_c, wf2_c, bq_c, bk_c, bu_c, bv_c,
         atab_c) = gdata[g]
        m = {
            "qT": np.ascontiguousarray(query[b].T).astype(npdt),
            "kT": np.ascontiguousarray(key[b].T).astype(npdt),
            "vT": np.ascontiguousarray(value[b].T).astype(npdt),
            "wq": wq_c, "wk": wk_c, "wv": wv_c, "wu": wu_c, "wf2": wf2_c,
            "bq": bq_c, "bk": bk_c, "bu": bu_c, "bv": bv_c, "atab": atab_c,
            "ones1": _ONES128,
        }
        mb = attn_mask[b]
        if not causal:
            import ml_dtypes as _mld
            mf = np.empty((128, NHL, S), _mld.bfloat16)
            for jb in range(8):
                mf[:, jb, :] = mb[:, jb * 128:(jb + 1) * 128].T
            m["maskf"] = mf
        in_maps.append(m)
    return in_maps


def _host_shards_fast(query, key, value, Wq, bq, Wk, bk, Wv, bv,
                      Wu, bu, Wf2, rel_table, mm_dt=None):
    """Per-core input maps for the fast causal build."""
    import ml_dtypes
    npdt = (np.dtype(ml_dtypes.bfloat16) if (mm_dt or MM_DT) == "bf16"
            else np.float32)
    e4np = ml_dtypes.float8_e4m3

    def to8(a):
        return np.clip(np.asarray(a, np.float32),
                       -240.0, 240.0).astype(e4np)

    in_maps = []
    gdata = []
    for g in range(HGRP):
        c0, c1 = g * NHL * HD, (g + 1) * NHL * HD
        wq8_c = to8(np.ascontiguousarray(Wq[:, c0:c1]) * AW)
        wk8_c = to8(np.ascontiguousarray(Wk[:, c0:c1]) * AW)
        wv_c = np.ascontiguousarray(Wv[:, c0:c1]).astype(npdt)
        wu_c = np.ascontiguousarray(Wu[:, c0:c1]).astype(npdt)
        wf2_c = np.ascontiguousarray(Wf2[c0:c1, :]).astype(npdt)
        bq_c = np.ascontiguousarray(bq[c0:c1].reshape(NHL, 128).T)
        bk_c = np.ascontiguousarray(bk[c0:c1].reshape(NHL, 128).T)
        bu_c = np.ascontiguousarray(bu[c0:c1].reshape(NHL, 128).T)
        bvb_c = np.ascontiguousarray(
            np.broadcast_to(bv[c0:c1][None, :], (128, NHL * HD))
        ).astype(ml_dtypes.bfloat16)
        # at2[h, r, y'] = table[y' + MAXLEN-1 - r]; entries with
        # y'+MAXLEN-1-r < MAXLEN-1 (key > query) are -1e5 -> silu = 0.
        y = np.arange(MAXLEN - 1, 2 * MAXLEN - 1)[None, :]   # [1, 1024]
        r = np.arange(128)[:, None]
        idx = y - r                       # [128, 1024], in [896, 2046]
        cols = rel_table[:, g * NHL:(g + 1) * NHL]   # [2047, NHL]
        cols = np.where(np.arange(2 * MAXLEN - 1)[:, None] >= MAXLEN - 1,
                        cols, np.float32(-1e5))
        at2_c = np.ascontiguousarray(
            cols[idx].transpose(2, 0, 1)).astype(ml_dtypes.bfloat16)
        gdata.append((wq8_c, wk8_c, wv_c, wu_c, wf2_c, bq_c, bk_c, bu_c,
                      bvb_c, at2_c))

    for c in range(NCORES):
        b, g = c // HGRP, c % HGRP
        (wq8_c, wk8_c, wv_c, wu_c, wf2_c, bq_c, bk_c, bu_c, bvb_c,
         at2_c) = gdata[g]
        qTb = np.ascontiguousarray(query[b].T)
        in_maps.append({
            "qT": qTb.astype(npdt),
            "qT8": to8(qTb),
            "kT8": to8(np.ascontiguousarray(key[b].T)),
            "vT": np.ascontiguousarray(value[b].T).astype(npdt),
            "wq8": wq8_c, "wk8": wk8_c, "wv": wv_c, "wu": wu_c,
            "wf2": wf2_c,
            "bq": bq_c, "bk": bk_c, "bu": bu_c, "bvb": bvb_c,
            "at2": at2_c,
        })
    return in_maps


def kernel(query, key, value, attn_mask, Wq, bq, Wk, bk, Wv, bv, Wu, bu,
           Wf2, bf2, rel_table):
    global LAST_EXEC_NS, LAST_RES
    query = np.asarray(query, np.float32)
    key = np.asarray(key, np.float32)
    value = np.asarray(value, np.float32)
    attn_mask = np.asarray(attn_mask, bool)
    Wq, bq = np.asarray(Wq, np.float32), np.asarray(bq, np.float32)
    Wk, bk = np.asarray(Wk, np.float32), np.asarray(bk, np.float32)
    Wv, bv = np.asarray(Wv, np.float32), np.asarray(bv, np.float32)
    Wu, bu = np.asarray(Wu, np.float32), np.asarray(bu, np.float32)
    Wf2, bf2 = np.asarray(Wf2, np.float32), np.asarray(bf2, np.float32)
    rel_table = np.asarray(rel_table, np.float32)

    tril = np.tril(np.ones((S, S), bool))
    causal = all(np.array_equal(attn_mask[b], tril) for b in range(B))

    key_ = (causal, MM_DT)
    if key_ not in _CACHE:
        _CACHE[key_] = _build_fast() if causal else _build(causal)
    nc = _CACHE[key_]

    if causal:
        in_maps = _host_shards_fast(query, key, value, Wq, bq, Wk, bk,
                                    Wv, bv, Wu, bu, Wf2, rel_table)
    else:
        in_maps = _host_shards(query, key, value, attn_mask, Wq, bq, Wk,
                               bk, Wv, bv, Wu, bu, Wf2, rel_table, causal)
    res = run_bass_kernel_spmd(nc, in_maps, list(range(NCORES)), trace=TRACE)
    LAST_RES = res
    if res.exec_time_ns is not None:
        LAST_EXEC_NS = res.exec_time_ns

    outp = np.empty((B, S, H), np.float32)
    for b in range(B):
        outp[b] = (np.asarray(res.results[2 * b]["out"], np.float32)
                   + np.asarray(res.results[2 * b + 1]["out"], np.float32)
                   + bf2[None, :])
    return outp

